# revision 2
# baseline (speedup 1.0000x reference)
"""ChemProp directed-MPNN forward pass on 8 Trainium2 NeuronCores (Bass/Tile).

v2: data-parallel over graphs with duplicated rev-closure edge compute.
Segment sums run as dma_scatter_add into HBM accumulator pairs (A/B per
fold, round-structured so each scatter call has distinct destination rows);
iter2 reuses the t=0 slot layout so its bias is a sequential read. Only the
two small node-sum arrays are AllGathered (bf16).
kernel(**inputs) -> np.float32 [5000, 256].
"""
import sys
sys.path.insert(0, "/opt/trn_rl_repo")
import numpy as np
import ml_dtypes
import concourse.bass as bass
import concourse.bacc as bacc
import concourse.mybir as mybir
import concourse.tile as tile
from concourse.library_config import mlp

N_NODES = 160000
N_EDGES = 640000
NUM_GRAPHS = 5000

bf16 = ml_dtypes.bfloat16

H = 256        # hidden
XD = 128       # node feature dim
ED = 64        # edge feature dim
TILE = 128


def _ceil(a, b):
    return -(-a // b)


def _rup(a, b):
    return _ceil(a, b) * b


def preprocess(inputs, G, NC=8, WIN=32768, CHUNK=2048):
    """Host-side index preprocessing. Returns (in_maps, meta, post)."""
    x = np.asarray(inputs["x"], np.float32)
    ei = np.asarray(inputs["edge_index"]).astype(np.int64)
    rev = np.asarray(inputs["revedge_index"]).astype(np.int64)
    ea = np.asarray(inputs["edge_attr"], np.float32)
    batch = np.asarray(inputs["batch"]).astype(np.int64)
    W1m = np.asarray(inputs["W1"], np.float32)
    W2m = np.asarray(inputs["W2"], np.float32)
    W3m = np.asarray(inputs["W3"], np.float32)
    b3 = np.asarray(inputs["b3"], np.float32)
    N = x.shape[0]
    E = ei.shape[1]
    src, dst = ei[0], ei[1]

    # ---- 1. graph -> core split by in-edge counts -------------------------
    edge_g = batch[dst]
    epg = np.bincount(edge_g, minlength=G)
    cum = np.concatenate([[0], np.cumsum(epg)])
    bounds = [int(np.searchsorted(cum, c * E / NC)) for c in range(NC + 1)]
    bounds[0], bounds[NC] = 0, G
    for c in range(1, NC):
        bounds[c] = min(max(bounds[c], bounds[c - 1]), G)
    gr = [(bounds[c], bounds[c + 1]) for c in range(NC)]
    nb = np.searchsorted(batch, bounds)  # [NC+1]
    core_of_node = np.zeros(N, np.int64)
    for c in range(NC):
        core_of_node[nb[c]:nb[c + 1]] = c

    # ---- 2. node slot space (graph-group padded, uniform boundaries) -----
    ngraphs = [gr[c][1] - gr[c][0] for c in range(NC)]
    GPAD = _rup(max(max(ngraphs), 1), TILE)
    NGG = GPAD // TILE
    ggcnt = np.zeros((NC, NGG), np.int64)
    for c in range(NC):
        glo, ghi = gr[c]
        for gg in range(NGG):
            g0, g1 = glo + gg * TILE, min(glo + (gg + 1) * TILE, ghi)
            if g0 >= ghi:
                continue
            n0 = np.searchsorted(batch, g0)
            n1 = np.searchsorted(batch, g1)
            ggcnt[c, gg] = n1 - n0
    ggpad = np.maximum(_rup(ggcnt.max(axis=0), TILE), TILE)
    ggoff = np.concatenate([[0], np.cumsum(ggpad)])
    S = int(ggoff[-1])
    NG = S // TILE
    NSROWS = NC * S
    NSW = _ceil(NSROWS, WIN)

    node_slot = np.full(N, -1, np.int64)
    for c in range(NC):
        glo, ghi = gr[c]
        for gg in range(NGG):
            g0, g1 = glo + gg * TILE, min(glo + (gg + 1) * TILE, ghi)
            if g0 >= ghi:
                continue
            n0 = np.searchsorted(batch, g0)
            n1 = np.searchsorted(batch, g1)
            node_slot[n0:n1] = ggoff[gg] + np.arange(n1 - n0)
    node_nsrow = core_of_node * S + node_slot
    sw_of_edge = node_nsrow[src] // WIN

    tile_gg = np.zeros(NG, np.int64)
    for gg in range(NGG):
        tile_gg[ggoff[gg] // TILE: ggoff[gg + 1] // TILE] = gg

    # ---- 3. per-core worklists -------------------------------------------
    cores = []
    for c in range(NC):
        E_c = np.nonzero(core_of_node[dst] == c)[0]
        R = rev[E_c]
        W1s = np.union1d(E_c, R)
        D = rev[W1s]
        W0s = np.union1d(W1s, D)
        t = np.full(E, -1, np.int8)
        t[W0s] = 2
        t[W1s] = 1
        t[E_c] = 0
        cores.append(dict(E_c=E_c, W0=W0s, t=t))

    # ---- 4. classes / supwins --------------------------------------------
    # class k = t*NSW + sw(src); supwins over slot space for g-gather windows.
    NCLS = 3 * NSW
    cls_of = []
    raw_cls = np.zeros((NC, NCLS), np.int64)
    for c in range(NC):
        W0s = cores[c]["W0"]
        t = cores[c]["t"][W0s].astype(np.int64)
        k = t * NSW + sw_of_edge[W0s]
        cls_arr = np.full(E, -1, np.int64)
        cls_arr[W0s] = k
        cls_of.append(cls_arr)
        raw_cls[c] = np.bincount(k, minlength=NCLS)

    # t=0 classes get round padding (est <= ~6 live rounds x ~12 buckets x 128);
    # t=1/2 only bucket padding. Post-hoc asserts below verify the bounds.
    SLACK = np.empty(NCLS, np.int64)
    SLACK[:NSW] = 11264
    SLACK[NSW:] = (NCLS + 2) * TILE
    cls_ub = _rup(raw_cls.max(axis=0), TILE) + SLACK
    assert (cls_ub <= WIN).all(), "class exceeds int16 window; need finer split"

    supwin_of_cls = np.zeros(NCLS, np.int64)
    sw_id, acc = 0, 0
    for k in range(NCLS):
        if (acc + cls_ub[k] > WIN and acc > 0) or (k % NSW == 0 and k > 0):
            sw_id += 1
            acc = 0
        supwin_of_cls[k] = sw_id
        acc += cls_ub[k]
    NSUP = sw_id + 1
    for s in range(NSUP):
        ks = np.nonzero(supwin_of_cls == s)[0]
        assert len(set(ks // NSW)) == 1, "supwin straddles t boundary"

    # ---- 5. bucket sizes --------------------------------------------------
    # t=0 buckets: (k<NSW, gw) with ROUND substructure (for dup-free scatter
    # calls: within a round every dst-slot is distinct).
    # t=1 buckets: (k, gw) single block.
    MAXR = 24
    raw_bkt1 = np.zeros((NC, NSW, NSUP), np.int64)      # t=1
    raw_rnd = np.zeros((NC, NSW, NSUP, MAXR), np.int64)  # t=0 rounds
    ecore_info = []
    for c in range(NC):
        W0s = cores[c]["W0"]
        t = cores[c]["t"][W0s]
        e1 = W0s[t == 1]
        k1 = cls_of[c][e1]
        gw1 = supwin_of_cls[cls_of[c][rev[e1]]]
        np.add.at(raw_bkt1, (np.full(len(e1), c), k1 - NSW, gw1), 1)
        e0 = W0s[t == 0]
        k0 = cls_of[c][e0]
        gw0 = supwin_of_cls[cls_of[c][rev[e0]]]
        # round = rank of edge within (bucket, dst)
        dstslot = node_slot[dst[e0]]
        order = np.lexsort((node_nsrow[src[e0]], dstslot, gw0, k0))
        eo, ko, gwo, do = e0[order], k0[order], gw0[order], dstslot[order]
        key = ((ko * NSUP + gwo) << 16) | do
        newgrp = np.concatenate([[True], key[1:] != key[:-1]])
        grp_first = np.nonzero(newgrp)[0]
        rnd = np.arange(len(eo)) - grp_first[np.cumsum(newgrp) - 1]
        assert rnd.max() < MAXR, f"round overflow {rnd.max()}"
        np.add.at(raw_rnd, (np.full(len(eo), c), ko, gwo, rnd), 1)
        ecore_info.append(dict(e0=eo, k0=ko, gw0=gwo, rnd=rnd, d0=do))

    rndpad = _rup(raw_rnd.max(axis=0), TILE)            # [NSW, NSUP, MAXR]
    bkt0pad = rndpad.sum(axis=2)                        # [NSW, NSUP]
    bkt1pad = _rup(raw_bkt1.max(axis=0), TILE)          # [NSW, NSUP]

    cls_final = np.zeros(NCLS, np.int64)
    cls_final[:NSW] = bkt0pad.sum(axis=1)
    cls_final[NSW:2 * NSW] = bkt1pad.sum(axis=1)
    cls_final[2 * NSW:] = _rup(raw_cls.max(axis=0)[2 * NSW:], TILE)
    cls_final = np.maximum(cls_final, TILE)
    assert (cls_final <= cls_ub).all(), "round padding blew the class ub"
    cls_off = np.concatenate([[0], np.cumsum(cls_final)])
    O0pad = int(cls_off[NCLS])
    O1E = int(cls_off[NSW])
    O1pad = int(cls_off[2 * NSW])
    supwin_base = np.array([int(cls_off[np.nonzero(supwin_of_cls == s)[0][0]])
                            for s in range(NSUP)])
    for s in range(NSUP):
        ks = np.nonzero(supwin_of_cls == s)[0]
        span = cls_off[ks[-1] + 1] - supwin_base[s]
        assert span <= WIN, f"supwin {s} span {span}"

    # bucket/round offsets
    bkt0_off = np.zeros((NSW, NSUP), np.int64)
    rnd_off = np.zeros((NSW, NSUP, MAXR), np.int64)
    for k in range(NSW):
        b = cls_off[k]
        for w in range(NSUP):
            bkt0_off[k, w] = b
            ro = b
            for r in range(MAXR):
                rnd_off[k, w, r] = ro
                ro += rndpad[k, w, r]
            b = ro
    bkt1_off = np.zeros((NSW, NSUP), np.int64)
    for k in range(NSW):
        b = cls_off[NSW + k]
        for w in range(NSUP):
            bkt1_off[k, w] = b
            b += bkt1pad[k, w]

    # ---- 6. per-core slot assignment -------------------------------------
    slot_edge = np.full((NC, O0pad), -1, np.int64)
    pos0 = np.full((NC, E), -1, np.int64)
    for c in range(NC):
        inf = ecore_info[c]
        eo, ko, gwo, rnd = inf["e0"], inf["k0"], inf["gw0"], inf["rnd"]
        # rank within (k, gw, rnd) in the sorted order (already grouped)
        key = (((ko * NSUP + gwo) * MAXR + rnd) << 1)
        order2 = np.lexsort((np.arange(len(eo)), key))
        eo2, ko2, gwo2, rnd2 = eo[order2], ko[order2], gwo[order2], rnd[order2]
        key2 = (ko2 * NSUP + gwo2) * MAXR + rnd2
        newg = np.concatenate([[True], key2[1:] != key2[:-1]])
        gf = np.nonzero(newg)[0]
        rank = np.arange(len(eo2)) - gf[np.cumsum(newg) - 1]
        slots = rnd_off[ko2, gwo2, rnd2] + rank
        slot_edge[c, slots] = eo2
        pos0[c, eo2] = slots
        # t=1
        W0s = cores[c]["W0"]
        t = cores[c]["t"][W0s]
        e1 = W0s[t == 1]
        k1 = cls_of[c][e1] - NSW
        gw1 = supwin_of_cls[cls_of[c][rev[e1]]]
        order = np.lexsort((node_nsrow[src[e1]], gw1, k1))
        e1o, k1o, gw1o = e1[order], k1[order], gw1[order]
        keyp = k1o * NSUP + gw1o
        newg = np.concatenate([[True], keyp[1:] != keyp[:-1]])
        gf = np.nonzero(newg)[0]
        rank = np.arange(len(e1o)) - gf[np.cumsum(newg) - 1]
        slots = bkt1_off[k1o, gw1o] + rank
        slot_edge[c, slots] = e1o
        pos0[c, e1o] = slots
        # t=2
        e2 = W0s[t == 2]
        k2 = cls_of[c][e2]
        order = np.lexsort((node_nsrow[src[e2]], k2))
        e2o, k2o = e2[order], k2[order]
        newg = np.concatenate([[True], k2o[1:] != k2o[:-1]])
        gf = np.nonzero(newg)[0]
        rank = np.arange(len(e2o)) - gf[np.cumsum(newg) - 1]
        slots = cls_off[k2o] + rank
        slot_edge[c, slots] = e2o
        pos0[c, e2o] = slots

    # ---- 7. call tables ---------------------------------------------------
    def region_of(slotbase):
        if slotbase < O1E:
            return "E", slotbase
        if slotbase < O1pad:
            return "R", slotbase - O1E
        return "D", slotbase - O1pad

    # it1 t=0 calls == it2 calls == L1-E calls: per (k, gw) bucket, chunked.
    # Each call carries its scatter segments: (rel, n, tab) with rel
    # call-relative, segments = (round ∩ call), tab = round parity.
    it0_calls = []
    for k in range(NSW):
        for w in range(NSUP):
            n = int(bkt0pad[k, w])
            if n == 0:
                continue
            s0 = int(bkt0_off[k, w])
            greg, gb = region_of(int(supwin_base[w]))
            # round intervals within this bucket
            rounds = []
            for r in range(MAXR):
                rn = int(rndpad[k, w, r])
                if rn:
                    rounds.append((int(rnd_off[k, w, r]), rn, r & 1))
            for o in range(0, n, CHUNK):
                cn = min(CHUNK, n - o)
                c0 = s0 + o
                segs = []
                for ra, rn, tab in rounds:
                    a = max(ra, c0)
                    b = min(ra + rn, c0 + cn)
                    if a < b:
                        segs.append((a - c0, b - a, tab))
                it0_calls.append(dict(slot0=c0, n=cn, s_base=int(k * WIN),
                                      g_reg=greg, g_base=int(gb), segs=segs))
    it1b_calls = []
    for k in range(NSW):
        for w in range(NSUP):
            n = int(bkt1pad[k, w])
            if n == 0:
                continue
            s0 = int(bkt1_off[k, w])
            greg, gb = region_of(int(supwin_base[w]))
            for o in range(0, n, CHUNK):
                it1b_calls.append(dict(slot0=s0 + o, n=min(CHUNK, n - o),
                                       s_base=int(k * WIN), g_reg=greg,
                                       g_base=int(gb)))

    # L1 calls: E region reuses the it0 grid (so fold0 scatter segs apply);
    # R/D regions are class-chunked.
    l1_calls = [dict(slot0=c["slot0"], n=c["n"], p_base=c["s_base"],
                     segs=c["segs"]) for c in it0_calls]
    for k in range(NSW, NCLS):
        n = int(cls_final[k])
        s0 = int(cls_off[k])
        for o in range(0, n, CHUNK):
            l1_calls.append(dict(slot0=s0 + o, n=min(CHUNK, n - o),
                                 p_base=int((k % NSW) * WIN), segs=None))

    # ---- 8. stage per-core arrays ----------------------------------------
    def wrap_idx(flat):
        a = np.asarray(flat, np.int16).reshape(-1, 16).T
        return np.tile(a, (8, 1))

    xbf = x.astype(bf16)
    xg_T = np.zeros((128, NSROWS), bf16)
    for c in range(NC):
        nlo, nhi = nb[c], nb[c + 1]
        ns_slots = c * S + node_slot[nlo:nhi]
        xg_T[:, ns_slots] = xbf[nlo:nhi].T
    eabf = ea.astype(bf16)

    iota = np.tile(np.arange(128, dtype=np.float32).astype(bf16)[None, :], (128, 1))
    ident = np.eye(128, dtype=np.float32).astype(bf16)
    w1e = W1m[XD:].astype(bf16)
    w1x = W1m[:XD].astype(bf16)
    w2 = W2m.reshape(2, 128, H).transpose(1, 0, 2).astype(bf16)
    w3x = W3m[:XD].astype(bf16)
    w3v = W3m[XD:].reshape(2, 128, H).transpose(1, 0, 2).astype(bf16)
    b3row = b3.astype(bf16)[None, :]
    ones1 = np.ones((1, 128), bf16)

    TRASH = S  # scatter pad target row

    in_maps = []
    post = dict(gr=gr, S=S, GPAD=GPAD)
    for c in range(NC):
        se = slot_edge[c]
        valid = se >= 0
        sev = np.where(valid, se, 0)
        eaT = np.where(valid[None, :], eabf[sev].T, bf16(0))
        swslot = np.zeros(O0pad, np.int64)
        for k in range(NCLS):
            swslot[cls_off[k]:cls_off[k + 1]] = (k % NSW) * WIN
        idxP = np.where(valid, node_nsrow[src[sev]] - swslot, 0)
        idxP = np.clip(idxP, 0, WIN - 1)
        # it1 idx (over O1 slots)
        n1 = O1pad
        se1 = se[:n1]
        v1 = se1 >= 0
        se1v = np.where(v1, se1, 0)
        idx1s = np.where(v1, node_nsrow[src[se1v]] - swslot[:n1], 0)
        gsup = np.zeros(n1, np.int64)
        for k in range(NSW):
            for w in range(NSUP):
                a = bkt0_off[k, w]
                bnd = a + bkt0pad[k, w]
                gsup[a:bnd] = supwin_base[w]
                a2 = bkt1_off[k, w]
                bnd2 = a2 + bkt1pad[k, w]
                gsup[a2:bnd2] = supwin_base[w]
        idx1g = np.where(v1, pos0[c, rev[se1v]] - gsup, 0)
        idx1s = np.clip(idx1s, 0, WIN - 1)
        idx1g = np.clip(idx1g, 0, WIN - 1)
        # scatter dst idx over E region
        seE = se[:O1E]
        vE = seE >= 0
        sdstE = np.where(vE, node_slot[dst[np.where(vE, seE, 0)]], TRASH)
        # L3
        nlo, nhi = nb[c], nb[c + 1]
        xT_c = np.zeros((128, S), bf16)
        xT_c[:, node_slot[nlo:nhi]] = xbf[nlo:nhi].T
        dPool = np.full(S, -1.0, np.float32)
        glo = gr[c][0]
        lg = batch[nlo:nhi] - glo
        dPool[node_slot[nlo:nhi]] = (lg % TILE).astype(np.float32)
        dPool_t = dPool.reshape(-1, TILE).T.astype(bf16)

        in_maps.append(dict(
            xg_T=xg_T, xT_c=xT_c, eaT=np.ascontiguousarray(eaT),
            idxP=wrap_idx(idxP), idx1s=wrap_idx(idx1s), idx1g=wrap_idx(idx1g),
            sdstE=wrap_idx(sdstE),
            dPool=np.ascontiguousarray(dPool_t),
            w1e=w1e, w1x=w1x, w2=np.ascontiguousarray(w2),
            w3x=w3x, w3v=np.ascontiguousarray(w3v), b3row=b3row, ones1=ones1,
            iota=iota, ident=ident,
        ))

    meta = dict(
        NC=NC, WIN=WIN, CHUNK=CHUNK, S=S, NG=NG, GPAD=GPAD, NGG=NGG,
        NSROWS=NSROWS, NSW=NSW, NSUP=NSUP,
        O0pad=O0pad, O1E=O1E, O1pad=O1pad,
        l1_calls=l1_calls, it0_calls=it0_calls, it1b_calls=it1b_calls,
        tile_gg=tile_gg.tolist(),
        cls_off=cls_off.tolist(), supwin_base=supwin_base.tolist(),
    )
    return in_maps, meta, post


dt = mybir.dt
Alu = mybir.AluOpType
Act = mybir.ActivationFunctionType


def build(meta, repeat=1, timing_mode=False, stages=None):
    def on(name):
        return stages is None or name in stages

    NC = meta["NC"]
    WIN, CHUNK = meta["WIN"], meta["CHUNK"]
    S, NG, GPAD, NGG = meta["S"], meta["NG"], meta["GPAD"], meta["NGG"]
    NSROWS = meta["NSROWS"]
    O0pad, O1E, O1pad = meta["O0pad"], meta["O1E"], meta["O1pad"]
    tile_gg = meta["tile_gg"]
    ST = S + TILE  # scatter table rows (incl trash)

    nc = bacc.Bacc(None, target_bir_lowering=False)

    def din(name, shape, d):
        return nc.declare_dram_parameter(name, list(shape), d, isOutput=False)

    xg_T = din("xg_T", [128, NSROWS], dt.bfloat16)
    xT_c = din("xT_c", [128, S], dt.bfloat16)
    eaT = din("eaT", [64, O0pad], dt.bfloat16)
    idxP = din("idxP", [128, O0pad // 16], dt.int16)
    idx1s = din("idx1s", [128, O1pad // 16], dt.int16)
    idx1g = din("idx1g", [128, O1pad // 16], dt.int16)
    sdstE = din("sdstE", [128, O1E // 16], dt.int16)
    dPool = din("dPool", [128, NG], dt.bfloat16)
    w1e = din("w1e", [64, H], dt.bfloat16)
    w1x = din("w1x", [128, H], dt.bfloat16)
    w2 = din("w2", [128, 2, H], dt.bfloat16)
    w3x = din("w3x", [128, H], dt.bfloat16)
    w3v = din("w3v", [128, 2, H], dt.bfloat16)
    b3row = din("b3row", [1, H], dt.bfloat16)
    ones1 = din("ones1", [1, 128], dt.bfloat16)
    iota = din("iota", [128, 128], dt.bfloat16)
    ident = din("ident", [128, 128], dt.bfloat16)
    out = nc.declare_dram_parameter("out", [GPAD, H], dt.float32, isOutput=True)

    def reg_tensor_base(slot):
        if slot < O1E:
            return "E", slot
        if slot < O1pad:
            return "R", slot - O1E
        return "D", slot - O1pad

    with tile.TileContext(nc) as tc:
        with (
            tc.tile_pool(name="dram", bufs=1, space="DRAM") as dram,
            tc.tile_pool(name="const", bufs=1) as cpool,
            tc.tile_pool(name="idx", bufs=4) as ipool,
            tc.tile_pool(name="ring", bufs=2) as ring,
            tc.tile_pool(name="stage", bufs=2) as stg,
            tc.tile_pool(name="mwork", bufs=2) as mw,
            tc.tile_pool(name="psA", bufs=2, space="PSUM") as psA,
            tc.tile_pool(name="psT", bufs=2, space="PSUM") as psT,
            tc.tile_pool(name="psF", bufs=1, space="PSUM") as psF,
        ):
            nc.gpsimd.load_library(mlp)

            w1e_t = cpool.tile([64, H], dt.bfloat16)
            nc.sync.dma_start(out=w1e_t[:], in_=w1e[:])
            w1x_t = cpool.tile([128, H], dt.bfloat16)
            nc.sync.dma_start(out=w1x_t[:], in_=w1x[:])
            w2_t = cpool.tile([128, 2 * H], dt.bfloat16)
            nc.sync.dma_start(out=w2_t[:], in_=w2[:].rearrange("p a b -> p (a b)"))
            w3x_t = cpool.tile([128, H], dt.bfloat16)
            nc.sync.dma_start(out=w3x_t[:], in_=w3x[:])
            w3v_t = cpool.tile([128, 2 * H], dt.bfloat16)
            nc.sync.dma_start(out=w3v_t[:], in_=w3v[:].rearrange("p a b -> p (a b)"))
            b3_t = cpool.tile([1, H], dt.bfloat16)
            nc.sync.dma_start(out=b3_t[:], in_=b3row[:])
            ones_t = cpool.tile([1, 128], dt.bfloat16)
            nc.sync.dma_start(out=ones_t[:], in_=ones1[:])
            iota_t = cpool.tile([128, 128], dt.bfloat16)
            nc.sync.dma_start(out=iota_t[:], in_=iota[:])
            ident_t = cpool.tile([128, 128], dt.bfloat16)
            nc.sync.dma_start(out=ident_t[:], in_=ident[:])
            dPool_t = cpool.tile([128, NG], dt.bfloat16)
            nc.sync.dma_start(out=dPool_t[:], in_=dPool[:])
            idx1s_t = cpool.tile([128, O1pad // 16], dt.int16)
            nc.sync.dma_start(out=idx1s_t[:], in_=idx1s[:])
            idx1g_t = cpool.tile([128, O1pad // 16], dt.int16)
            nc.sync.dma_start(out=idx1g_t[:], in_=idx1g[:])
            sdstE_t = cpool.tile([128, O1E // 16], dt.int16)
            nc.sync.dma_start(out=sdstE_t[:], in_=sdstE[:])
            zero_t = cpool.tile([128, 16 * H], dt.bfloat16)
            nc.vector.memset(zero_t[:], 0.0)

            for _rep in range(repeat):
                P = dram.tile([NSROWS, H], dt.bfloat16)
                h0E = dram.tile([O1E, H], dt.bfloat16)
                h0R = dram.tile([max(O1pad - O1E, TILE), H], dt.bfloat16)
                h0D = dram.tile([max(O0pad - O1pad, TILE), H], dt.bfloat16)
                h1E = dram.tile([O1E, H], dt.bfloat16)
                h1R = dram.tile([max(O1pad - O1E, TILE), H], dt.bfloat16)
                nsacc = [[dram.tile([ST, H], dt.bfloat16, tag=f"ns{i}{ab}",
                                    name=f"nsacc{i}{ab}")
                          for ab in "ab"] for i in range(3)]
                ns0s = dram.tile([S, H], dt.bfloat16)
                ns1s = dram.tile([S, H], dt.bfloat16)
                ns2d = dram.tile([S, H], dt.bfloat16)
                ns0f = dram.tile([NSROWS, H], dt.bfloat16, addr_space="Shared")
                ns1f = dram.tile([NSROWS, H], dt.bfloat16, addr_space="Shared")
                hreg0 = {"E": (h0E, O1E), "R": (h0R, O1pad - O1E), "D": (h0D, O0pad - O1pad)}
                hreg1 = {"E": (h1E, O1E), "R": (h1R, O1pad - O1E)}

                # zero the 6 scatter accumulators (overlaps with P pass)
                for i in range(3):
                    for ab in range(2):
                        t_ = nsacc[i][ab]
                        for b in range(0, ST, 2048):
                            n = min(2048, ST - b)
                            nc.sync.dma_start(
                                out=t_[b:b + n, :].rearrange("(t p) d -> p t d", p=128),
                                in_=zero_t[:, :(n // 128) * H].rearrange("p (t d) -> p t d", d=H))

                # ---- P pass ----------------------------------------------------
                PC = 2048
                for b in range(0, NSROWS, PC) if on("P") else []:
                    n = min(PC, NSROWS - b)
                    xc = ring.tile([128, PC], dt.bfloat16, tag="xc")
                    nc.sync.dma_start(out=xc[:, :n], in_=xg_T[:, b:b + n])
                    pstage = stg.tile([128, (PC // 128) * H], dt.bfloat16, tag="hst")
                    for t0 in range(0, n // 128, 2):
                        pn = min(2, n // 128 - t0)
                        ps = psA.tile([128, 2 * H], dt.float32, tag="main")
                        for j in range(pn):
                            t = t0 + j
                            nc.tensor.matmul(out=ps[:, j * H:(j + 1) * H], lhsT=xc[:, t * 128:(t + 1) * 128],
                                             rhs=w1x_t[:], start=True, stop=True)
                        nc.scalar.activation(pstage[:, t0 * H:(t0 + pn) * H], ps[:, :pn * H], Act.Copy)
                    nc.sync.dma_start(
                        out=P[b:b + n, :].rearrange("(t p) d -> p t d", p=128),
                        in_=pstage[:, :(n // 128) * H].rearrange("p (t d) -> p t d", d=H))

                # ---- scatter helper -------------------------------------------
                def scatter_chunk(call, hstage, tabs):
                    """Scatter this chunk's hstage rows into the fold tables."""
                    s0, n = call["slot0"], call["n"]
                    segs = call.get("segs")
                    if not segs:
                        return
                    for rel, sn, tab in segs:
                        nc.gpsimd.dma_scatter_add(
                            out_ap=tabs[tab][:, :],
                            in_ap=hstage[:, (rel // 128) * H:((rel + sn) // 128) * H]
                                .rearrange("p (k d) -> p k d", d=H),
                            idxs_ap=sdstE_t[:, (s0 + rel) // 16:(s0 + rel + sn) // 16],
                            num_idxs=sn, num_idxs_reg=sn, elem_size=H,
                            single_packet=False)

                # ---- L1 --------------------------------------------------------
                def l1_do(call, do_scatter):
                    s0, n, pb = call["slot0"], call["n"], call["p_base"]
                    nt = n // 128
                    it = ipool.tile([128, CHUNK // 16], dt.int16, tag="idx")
                    nc.sync.dma_start(out=it[:, :n // 16], in_=idxP[:, s0 // 16:(s0 + n) // 16])
                    gP = ring.tile([128, (CHUNK // 128) * H], dt.bfloat16, tag="gs")
                    pw = min(WIN, NSROWS - pb)
                    nc.gpsimd.dma_gather(
                        out_ap=gP[:, :nt * H].rearrange("p (k d) -> p k d", d=H),
                        in_ap=P[pb:pb + pw, :], idxs_ap=it[:, :n // 16],
                        num_idxs=n, num_idxs_reg=n, elem_size=H, single_packet=False)
                    ec = ring.tile([64, CHUNK], dt.bfloat16, tag="ea")
                    nc.sync.dma_start(out=ec[:, :n], in_=eaT[:, s0:s0 + n])
                    hstage = stg.tile([128, (CHUNK // 128) * H], dt.bfloat16, tag="hst")
                    for t0 in range(0, nt, 2):
                        pn = min(2, nt - t0)
                        ps = psA.tile([128, 2 * H], dt.float32, tag="main")
                        for j in range(pn):
                            t = t0 + j
                            nc.tensor.matmul(out=ps[:, j * H:(j + 1) * H], lhsT=ec[:, t * 128:(t + 1) * 128],
                                             rhs=w1e_t[:], start=True, stop=True)
                        nc.vector.tensor_tensor(out=ps[:, :pn * H], in0=ps[:, :pn * H],
                                                in1=gP[:, t0 * H:(t0 + pn) * H], op=Alu.add)
                        nc.scalar.activation(hstage[:, t0 * H:(t0 + pn) * H], ps[:, :pn * H], Act.Relu)
                    reg, loc = reg_tensor_base(s0)
                    hbuf = hreg0[reg][0]
                    nc.sync.dma_start(
                        out=hbuf[loc:loc + n, :].rearrange("(t p) d -> p t d", p=128),
                        in_=hstage[:, :nt * H].rearrange("p (t d) -> p t d", d=H))
                    if do_scatter:
                        scatter_chunk(call, hstage, nsacc[0])

                for call in (meta["l1_calls"] if on("L1E") else []):
                    if call["slot0"] < O1E:
                        l1_do(call, True)

                # ---- sum pass: ns = A + B -------------------------------------
                def sum_tables(tabs, ns_out):
                    for b in range(0, S, 1024):
                        n = min(1024, S - b)
                        nt = n // 128
                        ta = ring.tile([128, 8 * H], dt.bfloat16, tag="sa")
                        nc.sync.dma_start(out=ta[:, :nt * H].rearrange("p (t d) -> p t d", d=H),
                                          in_=tabs[0][b:b + n, :].rearrange("(t p) d -> p t d", p=128))
                        tb = ring.tile([128, 8 * H], dt.bfloat16, tag="sb")
                        nc.sync.dma_start(out=tb[:, :nt * H].rearrange("p (t d) -> p t d", d=H),
                                          in_=tabs[1][b:b + n, :].rearrange("(t p) d -> p t d", p=128))
                        tsum = mw.tile([128, 8 * H], dt.bfloat16, tag="ssum")
                        nc.vector.tensor_tensor(out=tsum[:, :nt * H],
                                                in0=ta[:, :nt * H],
                                                in1=tb[:, :nt * H], op=Alu.add)
                        nc.sync.dma_start(
                            out=ns_out[b:b + n, :].rearrange("(t p) d -> p t d", p=128),
                            in_=tsum[:, :nt * H].rearrange("p (t d) -> p t d", d=H))

                if on("fold0"):
                    sum_tables(nsacc[0], ns0s)

                if timing_mode:
                    nc.sync.dma_start(out=ns0f[:S, :], in_=ns0s[:, :])
                else:
                    nc.gpsimd.collective_compute(
                        "AllGather", Alu.bypass, replica_groups=[list(range(NC))],
                        ins=[ns0s[:, :].opt()], outs=[ns0f[:, :].opt()])

                for call in (meta["l1_calls"] if on("L1RD") else []):
                    if call["slot0"] >= O1E:
                        l1_do(call, False)

                # ---- iter pass helper -----------------------------------------
                def iter_pass(calls, sfull, hreg, bias_hreg, out_to, fold_tabs):
                    for call in calls:
                        s0, n = call["slot0"], call["n"]
                        nt = n // 128
                        # bias: sequential read from bias_hreg at same slots
                        reg, loc = reg_tensor_base(s0)
                        hb = ring.tile([128, (CHUNK // 128) * H], dt.bfloat16, tag="hb")
                        nc.sync.dma_start(
                            out=hb[:, :nt * H].rearrange("p (t d) -> p t d", d=H),
                            in_=bias_hreg[reg][0][loc:loc + n, :].rearrange("(t p) d -> p t d", p=128))
                        hstage = stg.tile([128, (CHUNK // 128) * H], dt.bfloat16, tag="hst")
                        sbase = call["s_base"]
                        sw_rows = min(WIN, NSROWS - sbase)
                        gt, gtrows = hreg[call["g_reg"]]
                        gb = call["g_base"]
                        gw_rows = min(WIN, gtrows - gb)
                        gsT = ring.tile([128, 2 * CHUNK], dt.bfloat16, tag="gsT")
                        nc.gpsimd.dma_gather(
                            out_ap=gsT[:, :2 * n].rearrange("p (c n) -> p c n", n=n),
                            in_ap=sfull[sbase:sbase + sw_rows, :],
                            idxs_ap=idx1s_t[:, s0 // 16:(s0 + n) // 16],
                            num_idxs=n, num_idxs_reg=n, elem_size=H,
                            transpose=True, single_packet=False)
                        ggT = ring.tile([128, 2 * CHUNK], dt.bfloat16, tag="ggT")
                        nc.gpsimd.dma_gather(
                            out_ap=ggT[:, :2 * n].rearrange("p (c n) -> p c n", n=n),
                            in_ap=gt[gb:gb + gw_rows, :],
                            idxs_ap=idx1g_t[:, s0 // 16:(s0 + n) // 16],
                            num_idxs=n, num_idxs_reg=n, elem_size=H,
                            transpose=True, single_packet=False)
                        mT = mw.tile([128, 2 * CHUNK], dt.bfloat16, tag="mT")
                        nc.vector.tensor_tensor(out=mT[:, :2 * n], in0=gsT[:, :2 * n],
                                                in1=ggT[:, :2 * n], op=Alu.subtract)
                        for t0 in range(0, nt, 2):
                            pn = min(2, nt - t0)
                            ps = psA.tile([128, 2 * H], dt.float32, tag="main")
                            for j in range(pn):
                                t = t0 + j
                                nc.tensor.matmul(out=ps[:, j * H:(j + 1) * H],
                                                 lhsT=mT[:, t * 128:t * 128 + 128],
                                                 rhs=w2_t[:, 0:H], start=True, stop=False)
                                nc.tensor.matmul(out=ps[:, j * H:(j + 1) * H],
                                                 lhsT=mT[:, n + t * 128:n + t * 128 + 128],
                                                 rhs=w2_t[:, H:2 * H], start=False, stop=True)
                            nc.vector.tensor_tensor(
                                out=ps[:, :pn * H], in0=ps[:, :pn * H],
                                in1=hb[:, t0 * H:(t0 + pn) * H], op=Alu.add)
                            nc.scalar.activation(hstage[:, t0 * H:(t0 + pn) * H],
                                                 ps[:, :pn * H], Act.Relu)
                        if out_to is not None:
                            reg, loc, obuf = out_to(call)
                            nc.sync.dma_start(
                                out=obuf[loc:loc + n, :].rearrange("(t p) d -> p t d", p=128),
                                in_=hstage[:, :nt * H].rearrange("p (t d) -> p t d", d=H))
                        if fold_tabs is not None:
                            scatter_chunk(call, hstage, fold_tabs)

                def out1(call):
                    s0 = call["slot0"]
                    reg, loc = reg_tensor_base(s0)
                    return reg, loc, hreg1[reg][0]

                if on("it1a"):
                    iter_pass(meta["it0_calls"], ns0f, hreg0, hreg0, out1, nsacc[1])

                if on("fold1"):
                    sum_tables(nsacc[1], ns1s)

                if timing_mode:
                    nc.sync.dma_start(out=ns1f[:S, :], in_=ns1s[:, :])
                else:
                    nc.gpsimd.collective_compute(
                        "AllGather", Alu.bypass, replica_groups=[list(range(NC))],
                        ins=[ns1s[:, :].opt()], outs=[ns1f[:, :].opt()])

                if on("it1b"):
                    iter_pass(meta["it1b_calls"], ns0f, hreg0, hreg0, out1, None)

                if on("it2"):
                    iter_pass(meta["it0_calls"], ns1f, hreg1, hreg0, None, nsacc[2])

                if on("fold2"):
                    sum_tables(nsacc[2], ns2d)

                # ---- L3 + pooling ---------------------------------------------
                gg_first = {}
                gg_last = {}
                for ti in range(NG):
                    gg = tile_gg[ti]
                    if gg not in gg_first:
                        gg_first[gg] = ti
                    gg_last[gg] = ti
                XC = 2048
                psG = None
                for b in range(0, S, XC) if on("L3") else []:
                    nb_ = min(XC, S - b)
                    xc = ring.tile([128, XC], dt.bfloat16, tag="xc")
                    nc.sync.dma_start(out=xc[:, :nb_], in_=xT_c[:, b:b + nb_])
                    for t in range(nb_ // 128):
                        ti = (b + t * 128) // 128
                        v = ring.tile([128, H], dt.bfloat16, tag="v")
                        nc.sync.dma_start(out=v[:], in_=ns2d[ti * 128:(ti + 1) * 128, :])
                        vT = mw.tile([128, H], dt.bfloat16, tag="mT")
                        for k in range(2):
                            pt = psT.tile([128, 128], dt.bfloat16, tag="tr")
                            nc.tensor.transpose(out=pt[:], in_=v[:, k * 128:(k + 1) * 128],
                                                identity=ident_t[:])
                            nc.vector.tensor_copy(out=vT[:, k * 128:(k + 1) * 128], in_=pt[:])
                        ps = psA.tile([128, H], dt.float32, tag="main")
                        nc.tensor.matmul(out=ps[:], lhsT=xc[:, t * 128:(t + 1) * 128],
                                         rhs=w3x_t[:], start=True, stop=False)
                        nc.tensor.matmul(out=ps[:], lhsT=vT[:, 0:128], rhs=w3v_t[:, 0:H],
                                         start=False, stop=False)
                        nc.tensor.matmul(out=ps[:], lhsT=vT[:, 128:256], rhs=w3v_t[:, H:2 * H],
                                         start=False, stop=False)
                        nc.tensor.matmul(out=ps[:], lhsT=ones_t[:], rhs=b3_t[:],
                                         start=False, stop=True)
                        na = mw.tile([128, H], dt.bfloat16, tag="na")
                        nc.scalar.activation(na[:], ps[:], Act.Relu)
                        gg = tile_gg[ti]
                        sel = mw.tile([128, 128], dt.bfloat16, tag="sel")
                        nc.vector.tensor_tensor(
                            out=sel[:], in0=dPool_t[:, ti:ti + 1].to_broadcast([128, 128]),
                            in1=iota_t[:], op=Alu.is_equal)
                        if ti == gg_first[gg]:
                            psG = psF.tile([128, H], dt.float32, tag="fold0")
                        nc.tensor.matmul(out=psG[:], lhsT=sel[:], rhs=na[:],
                                         start=(ti == gg_first[gg]), stop=(ti == gg_last[gg]))
                        if ti == gg_last[gg]:
                            ostage = stg.tile([128, H], dt.float32, tag="ost")
                            nc.vector.tensor_copy(out=ostage[:], in_=psG[:])
                            nc.sync.dma_start(out=out[gg * 128:(gg + 1) * 128, :], in_=ostage[:])

    nc.compile()
    return nc


import time
import jax
from jax.sharding import Mesh, PartitionSpec
from jax.experimental.shard_map import shard_map
from concourse import bass2jax
from concourse.bass2jax import _bass_exec_p, install_neuronx_cc_hook


def make_runner(nc, in_maps, n_cores=8):
    install_neuronx_cc_hook()
    partition_name = nc.partition_id_tensor.name if nc.partition_id_tensor else None
    in_names, out_names, out_avals, zero_outs = [], [], [], []
    for alloc in nc.m.functions[0].allocations:
        if not isinstance(alloc, mybir.MemoryLocationSet):
            continue
        name = alloc.memorylocations[0].name
        if alloc.kind == "ExternalInput":
            if name != partition_name:
                in_names.append(name)
        elif alloc.kind == "ExternalOutput":
            out_names.append(name)
            shape = tuple(alloc.tensor_shape)
            dtype = mybir.dt.np(alloc.dtype)
            out_avals.append(jax.core.ShapedArray(shape, dtype))
            zero_outs.append(np.zeros(shape, dtype))
    n_params = len(in_names)
    all_in = list(in_names) + list(out_names)
    if partition_name is not None:
        all_in.append(partition_name)

    def _body(*args):
        operands = list(args)
        if partition_name is not None:
            operands.append(bass2jax.partition_id_tensor())
        outs = _bass_exec_p.bind(
            *operands,
            out_avals=tuple(out_avals),
            in_names=tuple(all_in),
            out_names=tuple(out_names),
            lowering_input_output_aliases=(),
            sim_require_finite=True,
            sim_require_nnan=True,
            nc=nc,
        )
        return tuple(outs)

    devices = jax.devices()[:n_cores]
    mesh = Mesh(np.asarray(devices), ("core",))
    n_outs = len(out_names)
    in_specs = (PartitionSpec("core"),) * (n_params + n_outs)
    out_specs = (PartitionSpec("core"),) * n_outs
    fn = jax.jit(shard_map(_body, mesh=mesh, in_specs=in_specs,
                           out_specs=out_specs, check_rep=False), keep_unused=True)

    sharding = jax.sharding.NamedSharding(mesh, PartitionSpec("core"))
    dev_in = []
    for i, name in enumerate(in_names):
        cat = np.concatenate([np.asarray(in_maps[c][name]) for c in range(n_cores)], axis=0)
        dev_in.append(jax.device_put(cat, sharding))
    for z in zero_outs:
        cat = np.zeros((n_cores * z.shape[0], *z.shape[1:]), z.dtype)
        dev_in.append(jax.device_put(cat, sharding))

    def run():
        outs = fn(*dev_in)
        jax.block_until_ready(outs)
        return outs

    def results(outs):
        return [
            {name: np.asarray(outs[i]).reshape(n_cores, *out_avals[i].shape)[c]
             for i, name in enumerate(out_names)}
            for c in range(n_cores)
        ]
    return run, results


def kernel(**inputs):
    in_maps, meta, post = preprocess(inputs, NUM_GRAPHS, NC=8, WIN=32768, CHUNK=2048)
    nc = build(meta)
    run, results = make_runner(nc, in_maps, 8)
    res = results(run())
    G = NUM_GRAPHS
    full = np.zeros((G, 256), np.float32)
    for c in range(8):
        glo, ghi = post["gr"][c]
        full[glo:ghi] = res[c]["out"][:ghi - glo]
    return full
